# revision 23
# baseline (speedup 1.0000x reference)
"""CTRNN cell (6 Euler unfolds) on 8 Trainium2 NeuronCores.

Math (per unfold, 6x):
    f     = tanh([x, s] @ W + b)
    s_new = s + 0.1 * (-s + f)  = 0.9*s + 0.1*f

Strategy (v6):
  - Data-parallel over batch: B=8192 -> 1024 rows/core, no cross-core
    communication. Host does the cheap numpy transposes/packing.
  - Transposed on-chip layout (features on SBUF partitions, batch on the
    free dim): W slices are directly the stationary lhsT, batch is the
    moving free dim.
  - ALL matmul operands fp16 (1.8e-3 rel err vs the 2e-2 gate): fp16
    streams at 1 col/cycle (f32 runs the half-rate FP32-HIGH path),
    halves every DMA byte, and the DVE tensor_tensor gets the 2x 16-bit
    mode.
  - Delta form: psum holds z_k = x@Wt + (10 s0)@(0.1 Wb) + sum tmp_i@wb
    across all unfolds, never restarted.  PSUM is EIGHT (128,512) tiles,
    one per (m-tile, chunk) = one 2KB bank each: the Tile dep tracker
    works at tile granularity, so per-(m,c) tiles let chunk c0's tanh
    run while chunk c1's matmul block is still on the PE (a shared
    (128,1024) tile serializes tanh behind BOTH chunks).
  - State is never materialized: tmp_k = f_k - s_k obeys
        tmp_{k+1} = f_{k+1} + u_k,   u_k = 0.9*tmp_k - f_k,
    so the critical op between tanh and the next matmul round is ONE
    2x-mode tensor_tensor add per chunk. The u pass runs full-width
    (128,1024) off the critical path, split to fit the round budget:
    m0/m1 as one DVE scalar_tensor_tensor (1x), m2/m3 as DVE
    tensor_scalar_mul (4x) + GpSimd tensor_tensor (Pool has no STT and
    its TENSOR_SCALAR is a 15us software fallback).
  - Final unfold computes nothing on-chip: f_5 and u_4 stream out as
    fp16 and the host finishes out = 0.1*f5 - 0.9*u4 (u_4 DMAs overlap
    the last matmul round; the tail is one half-chunk tanh chain + DMA).
  - Two batch-chunk streams (512 cols) interleave; round 0 runs j-outer
    (matches DMA-arrival order).
  - All input DMAs ride ONE HWDGE ring (sync/SP: the fast starter) in
    exact need order -- concurrent queues share the 16 SDMA engines
    round-robin, which starves the critical bytes. W and x split fine
    so the first-matmul gate is only 512KB.
  - Junk warm-up matmuls (N=256) span the load so the HAM clock gate is
    at 8/8 when the first real matmul issues.
"""

import numpy as np

UNFOLDS = 6
B, D, N = 8192, 512, 512
NCORES = 8
BC = B // NCORES          # batch rows per core
CH = 512                  # chunk = matmul moving free dim (PSUM bank)
NCH = BC // CH            # 2
P = 128
KT = D // P               # 4 k-tiles for x (and for s / tmp)
MT = N // P               # 4 m-tiles of the output dim
NJUNK = 30                # warm-up matmuls (N=256) spanning the load

_compiled_nc = None


def _build_nc():
    import concourse.bass as bass  # noqa: F401
    import concourse.bacc as bacc
    import concourse.tile as tile
    from concourse import mybir

    f32 = mybir.dt.float32
    f16 = mybir.dt.float16
    MULT = mybir.AluOpType.mult
    ADD = mybir.AluOpType.add
    SUB = mybir.AluOpType.subtract
    TANH = mybir.ActivationFunctionType.Tanh

    nc = bacc.Bacc("TRN2", target_bir_lowering=False, debug=False)

    xP = nc.dram_tensor("xP", [P, KT * BC], f16, kind="ExternalInput").ap()
    sP = nc.dram_tensor("sP", [P, KT * BC], f16, kind="ExternalInput").ap()
    wtP = nc.dram_tensor("wtP", [P, KT * N], f16, kind="ExternalInput").ap()
    wbP = nc.dram_tensor("wbP", [P, KT * N], f16, kind="ExternalInput").ap()
    bias = nc.dram_tensor("bias", [N], f32, kind="ExternalInput").ap()
    f5T = nc.dram_tensor("f5T", [N, BC], f16, kind="ExternalOutput").ap()
    u4T = nc.dram_tensor("u4T", [N, BC], f16, kind="ExternalOutput").ap()

    with tile.TileContext(nc) as tc:
        with (
            tc.tile_pool(name="weights", bufs=1) as wpool,
            tc.tile_pool(name="data", bufs=1) as data,
            tc.tile_pool(name="fpool", bufs=2) as fpool,
            tc.tile_pool(name="tpool", bufs=2) as tpool,
            tc.tile_pool(name="upool", bufs=2) as upool,
            tc.tile_pool(name="qpool", bufs=2) as qpool,
            tc.tile_pool(name="psum", bufs=1, space="PSUM") as psump,
        ):
            junk = wpool.tile([P, 256], f16, tag="junk", name="junk")
            nc.gpsimd.memset(junk[:], 0)

            # ---- all inputs on the sync HWDGE ring, in need order ---------
            wt_mega = wpool.tile([P, KT * N], f16, tag="wt", name="wt_mega")
            x_mega = data.tile([P, KT * BC], f16, tag="xm", name="x_mega")
            s_mega = data.tile([P, KT * BC], f16, tag="sm", name="s_mega")
            wb_mega = wpool.tile([P, KT * N], f16, tag="wb", name="wb_mega")
            bias_sb = wpool.tile([P, MT], f32, tag="bias", name="bias_sb")

            WH = KT * N // 2
            HALF = KT * BC // 2
            nc.sync.dma_start(wt_mega[:, 0:WH], wtP[:, 0:WH])   # Wt j0,j1
            nc.sync.dma_start(x_mega[:, 0:HALF], xP[:, 0:HALF])  # x j0,j1
            nc.sync.dma_start(wt_mega[:, WH:], wtP[:, WH:])     # Wt j2,j3
            nc.sync.dma_start(x_mega[:, HALF:], xP[:, HALF:])   # x j2,j3
            nc.sync.dma_start(s_mega[:, 0:HALF], sP[:, 0:HALF])
            nc.sync.dma_start(s_mega[:, HALF:], sP[:, HALF:])
            nc.sync.dma_start(wb_mega[:], wbP[:, :])
            nc.sync.dma_start(bias_sb[:], bias.rearrange("(m p) -> p m", p=P))

            wt = [wt_mega[:, j * N:(j + 1) * N] for j in range(KT)]
            wb = [wb_mega[:, j * N:(j + 1) * N] for j in range(KT)]
            x_sb = [x_mega[:, j * BC:(j + 1) * BC] for j in range(KT)]
            s_sb = [s_mega[:, j * BC:(j + 1) * BC] for j in range(KT)]

            # one PSUM bank per (m, chunk) so chunk streams don't serialize
            ps = [[psump.tile([P, CH], f32, tag=f"ps{m}_{c}",
                              name=f"ps{m}_{c}") for c in range(NCH)]
                  for m in range(MT)]

            # HAM warm-up: keep the PE busy while inputs stream in so the
            # first real matmul runs at 2.4 GHz.
            for r in range(NJUNK):
                nc.tensor.matmul(
                    ps[r % MT][0][:, 0:256],
                    lhsT=junk[:, 0:P], rhs=junk[:, 0:256],
                    start=True, stop=True, skip_group_check=True,
                )

            # round 0: z = x@Wt + s10@wb, j-outer (matches DMA arrival)
            for j, (w, rhs) in enumerate(
                    [(wt[i], x_sb[i]) for i in range(KT)]
                    + [(wb[i], s_sb[i]) for i in range(KT)]):
                for c in range(NCH):
                    cs = c * CH
                    for m in range(MT):
                        nc.tensor.matmul(
                            ps[m][c][:],
                            lhsT=w[:, m * P:(m + 1) * P],
                            rhs=rhs[:, cs:cs + CH],
                            start=(j == 0), stop=False,
                            skip_group_check=True,
                        )

            # s0 = 0.1*s10 on the idle DVE during round 0, so unfold 0's
            # tmp0 = f0 - s0 is a 2x tensor_tensor instead of a 1x STT
            s0_t = []
            for m in range(MT):
                s0 = data.tile([P, BC], f16, tag=f"s0_{m}", name=f"s0_{m}")
                for c in range(NCH):
                    cs = c * CH
                    nc.vector.tensor_scalar_mul(
                        s0[:, cs:cs + CH], s_sb[m][:, cs:cs + CH], 0.1)
                s0_t.append(s0)

            # ---- unfolds: interleaved chunk streams -----------------------
            # f/tmp/u tiles are full-width (128,1024) per m; ACT and the
            # critical tt write per-chunk halves, the u pass reads/writes
            # full width.
            f_t = [None] * MT
            tmp_t = [None] * MT
            u_t = [None] * MT
            for k in range(UNFOLDS):
                last = k == UNFOLDS - 1
                # phase 1: tanh + critical tmp per chunk half
                fs, ts_ = [None] * MT, [None] * MT
                u_new = [None] * MT
                for c in range(NCH):
                    cs = c * CH
                    for m in range(MT):
                        if c == 0:
                            fs[m] = fpool.tile([P, BC], f16, tag=f"f{m}",
                                               name=f"f{k}_{m}")
                            if not last:
                                ts_[m] = tpool.tile([P, BC], f16,
                                                    tag=f"t{m}",
                                                    name=f"t{k}_{m}")
                        f = fs[m]
                        nc.scalar.activation(
                            f[:, cs:cs + CH], ps[m][c][:], TANH,
                            bias=bias_sb[:, m:m + 1], scale=1.0,
                        )
                        if last:
                            # stream f5 out per half; host finishes
                            eng = nc.gpsimd if c == 0 else nc.sync
                            eng.dma_start(
                                f5T[m * P:(m + 1) * P, cs:cs + CH],
                                f[:, cs:cs + CH])
                            continue
                        t = ts_[m]
                        if k == 0:
                            # tmp0 = f0 - s0   (2x-mode tt)
                            nc.vector.tensor_tensor(
                                t[:, cs:cs + CH], f[:, cs:cs + CH],
                                s0_t[m][:, cs:cs + CH], SUB,
                            )
                        else:
                            # tmp_k = f_k + u_{k-1}   (2x-mode tt)
                            nc.vector.tensor_tensor(
                                t[:, cs:cs + CH], f[:, cs:cs + CH],
                                u_t[m][:, cs:cs + CH], ADD,
                            )
                    # k=4: u4 per chunk half, so u4(c0) DMAs out while the
                    # last matmul round still runs (the host needs u4 last)
                    for m in range(MT) if k == UNFOLDS - 2 else ():
                        if c == 0:
                            u_new[m] = upool.tile([P, BC], f16,
                                                  tag=f"u{m}",
                                                  name=f"u{k}_{m}")
                        q = qpool.tile([P, CH], f16, tag=f"q{c}_{m}",
                                       name=f"q{k}_{c}_{m}")
                        nc.vector.tensor_scalar_mul(
                            q[:], ts_[m][:, cs:cs + CH], 0.9)
                        nc.vector.tensor_tensor(
                            u_new[m][:, cs:cs + CH], q[:],
                            fs[m][:, cs:cs + CH], SUB)
                        eng = nc.sync if c == 0 else nc.gpsimd
                        eng.dma_start(u4T[m * P:(m + 1) * P, cs:cs + CH],
                                      u_new[m][:, cs:cs + CH])
                if last:
                    break
                f_t, tmp_t = fs, ts_
                # phase 2 (k<4): u pass full width on DVE, off the critical
                # path: q = 0.9*tmp (4x tensor_scalar), u = q - f (2x tt).
                # GpSimd is NOT used: a concurrent Pool elementwise op
                # steals the shared SBUF port and slows DVE ops ~3.3x.
                for m in range(MT) if k < UNFOLDS - 2 else ():
                    u = upool.tile([P, BC], f16, tag=f"u{m}",
                                   name=f"u{k}_{m}")
                    q = qpool.tile([P, BC], f16, tag=f"qw{m}",
                                   name=f"qw{k}_{m}")
                    nc.vector.tensor_scalar_mul(q[:], tmp_t[m][:], 0.9)
                    nc.vector.tensor_tensor(u[:], q[:], f_t[m][:], SUB)
                    u_new[m] = u
                for m in range(MT):
                    u_t[m] = u_new[m]
                # phase 3: next matmul round, chunk-interleaved. The very
                # last block (R5, c1) runs m-outer: its tmp deps are long
                # ready, and each psum tile completing early lets the final
                # tanh chain overlap the matmul tail instead of trailing it.
                for c in range(NCH):
                    cs = c * CH
                    m_outer = k == UNFOLDS - 2 and c == NCH - 1
                    order = ([(j, m) for m in range(MT) for j in range(KT)]
                             if m_outer else
                             [(j, m) for j in range(KT) for m in range(MT)])
                    for j, m in order:
                        nc.tensor.matmul(
                            ps[m][c][:],
                            lhsT=wb[j][:, m * P:(m + 1) * P],
                            rhs=tmp_t[j][:, cs:cs + CH],
                            start=False,
                            stop=(k == UNFOLDS - 2 and j == KT - 1),
                            skip_group_check=True,
                        )

    nc.compile()
    return nc


def _get_nc():
    global _compiled_nc
    if _compiled_nc is None:
        _compiled_nc = _build_nc()
    return _compiled_nc


def make_in_maps(x, s, W, b):
    """Shard + pack host-side: everything fp16, (128, k*C) layouts with
    k-tiles side by side so per-partition DMA runs are 4KB contiguous."""
    xT = np.ascontiguousarray(x.T)            # (D, B) f32
    sT = np.ascontiguousarray(10.0 * s.T)     # (N, B) f32, pre-scaled
    wt = np.ascontiguousarray(
        W[:D].reshape(KT, P, N).transpose(1, 0, 2).reshape(P, -1)
    ).astype(np.float16)
    wb = np.ascontiguousarray(
        (0.1 * W[D:]).reshape(KT, P, N).transpose(1, 0, 2).reshape(P, -1)
    ).astype(np.float16)
    in_maps = []
    for c in range(NCORES):
        sl = slice(c * BC, (c + 1) * BC)
        xs = xT[:, sl].reshape(KT, P, BC).transpose(1, 0, 2).reshape(P, -1)
        ss = sT[:, sl].reshape(KT, P, BC).transpose(1, 0, 2).reshape(P, -1)
        in_maps.append({
            "xP": np.ascontiguousarray(xs).astype(np.float16),
            "sP": np.ascontiguousarray(ss).astype(np.float16),
            "wtP": wt,
            "wbP": wb,
            "bias": np.ascontiguousarray(b.astype(np.float32)),
        })
    return in_maps


def kernel(**inputs):
    from concourse.bass_utils import run_bass_kernel_spmd

    x = np.asarray(inputs["inputs"], dtype=np.float32)
    s = np.asarray(inputs["state"], dtype=np.float32)
    W = np.ascontiguousarray(np.asarray(inputs["W"], dtype=np.float32))
    b = np.ascontiguousarray(np.asarray(inputs["bias"], dtype=np.float32))

    in_maps = make_in_maps(x, s, W, b)
    nc = _get_nc()
    res = run_bass_kernel_spmd(nc, in_maps, list(range(NCORES))).results
    f5 = np.concatenate([res[c]["f5T"] for c in range(NCORES)], axis=1)
    u4 = np.concatenate([res[c]["u4T"] for c in range(NCORES)], axis=1)
    # s6 = f5 - 0.9*tmp5 = 0.1*f5 - 0.9*u4   (tmp5 = f5 + u4)
    outT = 0.1 * f5.astype(np.float32) - 0.9 * u4.astype(np.float32)
    out = np.ascontiguousarray(outT.T)
    return (out, out)


# revision 24
# speedup vs baseline: 1.0284x; 1.0284x over previous
"""CTRNN cell (6 Euler unfolds) on 8 Trainium2 NeuronCores.

Math (per unfold, 6x):
    f     = tanh([x, s] @ W + b)
    s_new = s + 0.1 * (-s + f)  = 0.9*s + 0.1*f

Strategy (v6):
  - Data-parallel over batch: B=8192 -> 1024 rows/core, no cross-core
    communication. Host does the cheap numpy transposes/packing.
  - Transposed on-chip layout (features on SBUF partitions, batch on the
    free dim): W slices are directly the stationary lhsT, batch is the
    moving free dim.
  - ALL matmul operands fp16 (1.8e-3 rel err vs the 2e-2 gate): fp16
    streams at 1 col/cycle (f32 runs the half-rate FP32-HIGH path),
    halves every DMA byte, and the DVE tensor_tensor gets the 2x 16-bit
    mode.
  - Delta form: psum holds z_k = x@Wt + (10 s0)@(0.1 Wb) + sum tmp_i@wb
    across all unfolds, never restarted.  PSUM is EIGHT (128,512) tiles,
    one per (m-tile, chunk) = one 2KB bank each: the Tile dep tracker
    works at tile granularity, so per-(m,c) tiles let chunk c0's tanh
    run while chunk c1's matmul block is still on the PE (a shared
    (128,1024) tile serializes tanh behind BOTH chunks).
  - State is never materialized: tmp_k = f_k - s_k obeys
        tmp_{k+1} = f_{k+1} + u_k,   u_k = 0.9*tmp_k - f_k,
    so the critical op between tanh and the next matmul round is ONE
    2x-mode tensor_tensor add per chunk. The u pass runs full-width
    (128,1024) off the critical path, split to fit the round budget:
    m0/m1 as one DVE scalar_tensor_tensor (1x), m2/m3 as DVE
    tensor_scalar_mul (4x) + GpSimd tensor_tensor (Pool has no STT and
    its TENSOR_SCALAR is a 15us software fallback).
  - Final unfold computes nothing on-chip: f_5 and u_4 stream out as
    fp16 and the host finishes out = 0.1*f5 - 0.9*u4 (u_4 DMAs overlap
    the last matmul round; the tail is one half-chunk tanh chain + DMA).
  - Two batch-chunk streams (512 cols) interleave; round 0 runs j-outer
    (matches DMA-arrival order).
  - All input DMAs ride ONE HWDGE ring (sync/SP: the fast starter) in
    exact need order -- concurrent queues share the 16 SDMA engines
    round-robin, which starves the critical bytes. W and x split fine
    so the first-matmul gate is only 512KB.
  - Junk warm-up matmuls (N=256) span the load so the HAM clock gate is
    at 8/8 when the first real matmul issues.
"""

import numpy as np

UNFOLDS = 6
B, D, N = 8192, 512, 512
NCORES = 8
BC = B // NCORES          # batch rows per core
CH = 512                  # chunk = matmul moving free dim (PSUM bank)
NCH = BC // CH            # 2
P = 128
KT = D // P               # 4 k-tiles for x (and for s / tmp)
MT = N // P               # 4 m-tiles of the output dim
NJUNK = 33                # warm-up matmuls (N=256) spanning the load

_compiled_nc = None


def _build_nc():
    import concourse.bass as bass  # noqa: F401
    import concourse.bacc as bacc
    import concourse.tile as tile
    from concourse import mybir

    f32 = mybir.dt.float32
    f16 = mybir.dt.float16
    MULT = mybir.AluOpType.mult
    ADD = mybir.AluOpType.add
    SUB = mybir.AluOpType.subtract
    TANH = mybir.ActivationFunctionType.Tanh

    nc = bacc.Bacc("TRN2", target_bir_lowering=False, debug=False)

    xP = nc.dram_tensor("xP", [P, KT * BC], f16, kind="ExternalInput").ap()
    sP = nc.dram_tensor("sP", [P, KT * BC], f16, kind="ExternalInput").ap()
    wtP = nc.dram_tensor("wtP", [P, KT * N], f16, kind="ExternalInput").ap()
    wbP = nc.dram_tensor("wbP", [P, KT * N], f16, kind="ExternalInput").ap()
    bias = nc.dram_tensor("bias", [N], f32, kind="ExternalInput").ap()
    f5T = nc.dram_tensor("f5T", [N, BC], f16, kind="ExternalOutput").ap()
    u4T = nc.dram_tensor("u4T", [N, BC], f16, kind="ExternalOutput").ap()

    with tile.TileContext(nc) as tc:
        with (
            tc.tile_pool(name="weights", bufs=1) as wpool,
            tc.tile_pool(name="data", bufs=1) as data,
            tc.tile_pool(name="fpool", bufs=2) as fpool,
            tc.tile_pool(name="tpool", bufs=2) as tpool,
            tc.tile_pool(name="upool", bufs=2) as upool,
            tc.tile_pool(name="qpool", bufs=2) as qpool,
            tc.tile_pool(name="psum", bufs=1, space="PSUM") as psump,
        ):
            junk = wpool.tile([P, 256], f16, tag="junk", name="junk")
            nc.gpsimd.memset(junk[:], 0)

            # ---- all inputs on the sync HWDGE ring, in need order ---------
            wt_mega = wpool.tile([P, KT * N], f16, tag="wt", name="wt_mega")
            x_mega = data.tile([P, KT * BC], f16, tag="xm", name="x_mega")
            s_mega = data.tile([P, KT * BC], f16, tag="sm", name="s_mega")
            wb_mega = wpool.tile([P, KT * N], f16, tag="wb", name="wb_mega")
            bias_sb = wpool.tile([P, MT], f32, tag="bias", name="bias_sb")

            WH = KT * N // 2
            HALF = KT * BC // 2
            nc.sync.dma_start(wt_mega[:, 0:WH], wtP[:, 0:WH])   # Wt j0,j1
            nc.sync.dma_start(x_mega[:, 0:HALF], xP[:, 0:HALF])  # x j0,j1
            nc.sync.dma_start(wt_mega[:, WH:], wtP[:, WH:])     # Wt j2,j3
            nc.sync.dma_start(x_mega[:, HALF:], xP[:, HALF:])   # x j2,j3
            nc.sync.dma_start(s_mega[:, 0:HALF], sP[:, 0:HALF])
            nc.sync.dma_start(s_mega[:, HALF:], sP[:, HALF:])
            nc.sync.dma_start(wb_mega[:], wbP[:, :])
            nc.sync.dma_start(bias_sb[:], bias.rearrange("(m p) -> p m", p=P))

            wt = [wt_mega[:, j * N:(j + 1) * N] for j in range(KT)]
            wb = [wb_mega[:, j * N:(j + 1) * N] for j in range(KT)]
            x_sb = [x_mega[:, j * BC:(j + 1) * BC] for j in range(KT)]
            s_sb = [s_mega[:, j * BC:(j + 1) * BC] for j in range(KT)]

            # one PSUM bank per (m, chunk) so chunk streams don't serialize
            ps = [[psump.tile([P, CH], f32, tag=f"ps{m}_{c}",
                              name=f"ps{m}_{c}") for c in range(NCH)]
                  for m in range(MT)]

            # HAM warm-up: keep the PE busy while inputs stream in so the
            # first real matmul runs at 2.4 GHz.
            for r in range(NJUNK):
                nc.tensor.matmul(
                    ps[r % MT][0][:, 0:256],
                    lhsT=junk[:, 0:P], rhs=junk[:, 0:256],
                    start=True, stop=True, skip_group_check=True,
                )

            # round 0: z = x@Wt + s10@wb, j-outer (matches DMA arrival)
            for j, (w, rhs) in enumerate(
                    [(wt[i], x_sb[i]) for i in range(KT)]
                    + [(wb[i], s_sb[i]) for i in range(KT)]):
                for c in range(NCH):
                    cs = c * CH
                    for m in range(MT):
                        nc.tensor.matmul(
                            ps[m][c][:],
                            lhsT=w[:, m * P:(m + 1) * P],
                            rhs=rhs[:, cs:cs + CH],
                            start=(j == 0), stop=False,
                            skip_group_check=True,
                        )

            # s0 = 0.1*s10 on the idle DVE during round 0, so unfold 0's
            # tmp0 = f0 - s0 is a 2x tensor_tensor instead of a 1x STT
            s0_t = []
            for m in range(MT):
                s0 = data.tile([P, BC], f16, tag=f"s0_{m}", name=f"s0_{m}")
                for c in range(NCH):
                    cs = c * CH
                    nc.vector.tensor_scalar_mul(
                        s0[:, cs:cs + CH], s_sb[m][:, cs:cs + CH], 0.1)
                s0_t.append(s0)

            # ---- unfolds: interleaved chunk streams -----------------------
            # f/tmp/u tiles are full-width (128,1024) per m; ACT and the
            # critical tt write per-chunk halves, the u pass reads/writes
            # full width.
            f_t = [None] * MT
            tmp_t = [None] * MT
            u_t = [None] * MT
            for k in range(UNFOLDS):
                last = k == UNFOLDS - 1
                # phase 1: tanh + critical tmp per chunk half
                fs, ts_ = [None] * MT, [None] * MT
                u_new = [None] * MT
                for c in range(NCH):
                    cs = c * CH
                    for m in range(MT):
                        if c == 0:
                            fs[m] = fpool.tile([P, BC], f16, tag=f"f{m}",
                                               name=f"f{k}_{m}")
                            if not last:
                                ts_[m] = tpool.tile([P, BC], f16,
                                                    tag=f"t{m}",
                                                    name=f"t{k}_{m}")
                        f = fs[m]
                        nc.scalar.activation(
                            f[:, cs:cs + CH], ps[m][c][:], TANH,
                            bias=bias_sb[:, m:m + 1], scale=1.0,
                        )
                        if last:
                            # stream f5 out per half; host finishes
                            eng = nc.gpsimd if c == 0 else nc.sync
                            eng.dma_start(
                                f5T[m * P:(m + 1) * P, cs:cs + CH],
                                f[:, cs:cs + CH])
                            continue
                        t = ts_[m]
                        if k == 0:
                            # tmp0 = f0 - s0   (2x-mode tt)
                            nc.vector.tensor_tensor(
                                t[:, cs:cs + CH], f[:, cs:cs + CH],
                                s0_t[m][:, cs:cs + CH], SUB,
                            )
                        else:
                            # tmp_k = f_k + u_{k-1}   (2x-mode tt)
                            nc.vector.tensor_tensor(
                                t[:, cs:cs + CH], f[:, cs:cs + CH],
                                u_t[m][:, cs:cs + CH], ADD,
                            )
                    # k=4: u4 per chunk half, so u4(c0) DMAs out while the
                    # last matmul round still runs (the host needs u4 last)
                    for m in range(MT) if k == UNFOLDS - 2 else ():
                        if c == 0:
                            u_new[m] = upool.tile([P, BC], f16,
                                                  tag=f"u{m}",
                                                  name=f"u{k}_{m}")
                        q = qpool.tile([P, CH], f16, tag=f"q{c}_{m}",
                                       name=f"q{k}_{c}_{m}")
                        nc.vector.tensor_scalar_mul(
                            q[:], ts_[m][:, cs:cs + CH], 0.9)
                        nc.vector.tensor_tensor(
                            u_new[m][:, cs:cs + CH], q[:],
                            fs[m][:, cs:cs + CH], SUB)
                        eng = nc.sync if c == 0 else nc.gpsimd
                        eng.dma_start(u4T[m * P:(m + 1) * P, cs:cs + CH],
                                      u_new[m][:, cs:cs + CH])
                if last:
                    break
                f_t, tmp_t = fs, ts_
                # phase 2 (k<4): u pass full width on DVE, off the critical
                # path: q = 0.9*tmp (4x tensor_scalar), u = q - f (2x tt).
                # GpSimd is NOT used: a concurrent Pool elementwise op
                # steals the shared SBUF port and slows DVE ops ~3.3x.
                for m in range(MT) if k < UNFOLDS - 2 else ():
                    u = upool.tile([P, BC], f16, tag=f"u{m}",
                                   name=f"u{k}_{m}")
                    q = qpool.tile([P, BC], f16, tag=f"qw{m}",
                                   name=f"qw{k}_{m}")
                    nc.vector.tensor_scalar_mul(q[:], tmp_t[m][:], 0.9)
                    nc.vector.tensor_tensor(u[:], q[:], f_t[m][:], SUB)
                    u_new[m] = u
                for m in range(MT):
                    u_t[m] = u_new[m]
                # phase 3: next matmul round, chunk-interleaved. The very
                # last block (R5, c1) runs m-outer: its tmp deps are long
                # ready, and each psum tile completing early lets the final
                # tanh chain overlap the matmul tail instead of trailing it.
                for c in range(NCH):
                    cs = c * CH
                    m_outer = k == UNFOLDS - 2 and c == NCH - 1
                    order = ([(j, m) for m in range(MT) for j in range(KT)]
                             if m_outer else
                             [(j, m) for j in range(KT) for m in range(MT)])
                    for j, m in order:
                        nc.tensor.matmul(
                            ps[m][c][:],
                            lhsT=wb[j][:, m * P:(m + 1) * P],
                            rhs=tmp_t[j][:, cs:cs + CH],
                            start=False,
                            stop=(k == UNFOLDS - 2 and j == KT - 1),
                            skip_group_check=True,
                        )

    nc.compile()
    return nc


def _get_nc():
    global _compiled_nc
    if _compiled_nc is None:
        _compiled_nc = _build_nc()
    return _compiled_nc


def make_in_maps(x, s, W, b):
    """Shard + pack host-side: everything fp16, (128, k*C) layouts with
    k-tiles side by side so per-partition DMA runs are 4KB contiguous."""
    xT = np.ascontiguousarray(x.T)            # (D, B) f32
    sT = np.ascontiguousarray(10.0 * s.T)     # (N, B) f32, pre-scaled
    wt = np.ascontiguousarray(
        W[:D].reshape(KT, P, N).transpose(1, 0, 2).reshape(P, -1)
    ).astype(np.float16)
    wb = np.ascontiguousarray(
        (0.1 * W[D:]).reshape(KT, P, N).transpose(1, 0, 2).reshape(P, -1)
    ).astype(np.float16)
    in_maps = []
    for c in range(NCORES):
        sl = slice(c * BC, (c + 1) * BC)
        xs = xT[:, sl].reshape(KT, P, BC).transpose(1, 0, 2).reshape(P, -1)
        ss = sT[:, sl].reshape(KT, P, BC).transpose(1, 0, 2).reshape(P, -1)
        in_maps.append({
            "xP": np.ascontiguousarray(xs).astype(np.float16),
            "sP": np.ascontiguousarray(ss).astype(np.float16),
            "wtP": wt,
            "wbP": wb,
            "bias": np.ascontiguousarray(b.astype(np.float32)),
        })
    return in_maps


def kernel(**inputs):
    from concourse.bass_utils import run_bass_kernel_spmd

    x = np.asarray(inputs["inputs"], dtype=np.float32)
    s = np.asarray(inputs["state"], dtype=np.float32)
    W = np.ascontiguousarray(np.asarray(inputs["W"], dtype=np.float32))
    b = np.ascontiguousarray(np.asarray(inputs["bias"], dtype=np.float32))

    in_maps = make_in_maps(x, s, W, b)
    nc = _get_nc()
    res = run_bass_kernel_spmd(nc, in_maps, list(range(NCORES))).results
    f5 = np.concatenate([res[c]["f5T"] for c in range(NCORES)], axis=1)
    u4 = np.concatenate([res[c]["u4T"] for c in range(NCORES)], axis=1)
    # s6 = f5 - 0.9*tmp5 = 0.1*f5 - 0.9*u4   (tmp5 = f5 + u4)
    outT = 0.1 * f5.astype(np.float32) - 0.9 * u4.astype(np.float32)
    out = np.ascontiguousarray(outT.T)
    return (out, out)
